# revision 8
# baseline (speedup 1.0000x reference)
"""Trainium2 Bass kernel for nn_CausalDecayMemory.

Reference computation (B=4, T=4096, D=512):
    q = x @ Wq.T ; k = x @ Wk.T ; v = x @ Wv.T
    scores[b,t,s] = q[b,t] . k[b,s]
    weights[t,s] = decay^max(s-t-1, 0) for s > t else 0   (anti-causal, decayed)
    retrieved = (scores * weights) @ v
    out = (retrieved @ Wo.T) * out_scale

Strategy: 8 cores = 4 batches x 2 sequence halves. Each core handles one
(batch, 2048-position half) with a 512-position lookahead halo, using a
RetNet-style chunked-decay recurrence over super-chunks of C=512:
    intra-chunk: masked attention with decay mask
    cross-chunk: retrieved_cross[t] = gamma^(C-1-i) * q_t @ S_c,
                 S_c = Z_{c+1} + gamma^C * S_{c+1},
                 Z_c = sum_j gamma^j k[cC+j] (x) v[cC+j]
For the graded regime (decay_logit=3 -> gamma^512 ~ 1.6e-11) the state
recurrence truncates to S_c = Z_{c+1} ("fast" path, halo = 1 chunk).  If
gamma is close enough to 1 that truncation would matter, a "general"
variant processes the full remaining tail with the exact recurrence.

All matmuls are K=128 x M=128 x N=512.  q is pre-scaled by gamma^(C-1-i)
and k by gamma^j, which makes the intra-chunk mask a constant
gamma^(-C) * strict-lower-triangular matrix and lets one scaled copy of
k/q serve both the intra and cross paths.
"""

import os
import sys

import numpy as np

for _p in ("/opt/trn_rl_repo",):
    if _p not in sys.path and os.path.isdir(_p):
        sys.path.insert(0, _p)

import concourse.bass as bass  # noqa: E402
import concourse.mybir as mybir  # noqa: E402
import concourse.tile as tile  # noqa: E402
from concourse import bacc  # noqa: E402
from concourse.bass_utils import run_bass_kernel_spmd  # noqa: E402

B, T, D = 4, 4096, 512
P = 128
C = 512          # super-chunk length
NS = 4           # 128-sub-tiles per 512
NL = 4           # local super-chunks per core (2048 positions)
N_CORES = 8

F32 = mybir.dt.float32
# Matmul input dtype: float32r streams 4x faster than float32 on the PE at
# N>=256 (single-pass relaxed-precision fp32); same bit layout as fp32.
USE_F32R = os.environ.get("KERNEL_F32", "") != "1"

_BUILD_CACHE: dict = {}
LAST_RESULTS = None  # BassKernelResults of the most recent run (for test.py)


MD = mybir.dt.float32r if USE_F32R else F32  # matmul-input dtype


def _build(NE: int, has_state: bool):
    """Build + compile the per-core Bass program. NE = total super-chunks
    (NL local + lookahead tail); has_state = carry decayed KV state across
    chunks (exact for any gamma) vs. single-chunk truncation."""
    key = (NE, has_state, USE_F32R)
    if key in _BUILD_CACHE:
        return _BUILD_CACHE[key]

    nc = bacc.Bacc("TRN2", target_bir_lowering=False, debug=False)

    xT = nc.dram_tensor("xT", [D, NE * C], MD, kind="ExternalInput").ap()
    wqT = nc.dram_tensor("wqT", [D, D], MD, kind="ExternalInput").ap()
    wkT = nc.dram_tensor("wkT", [D, D], MD, kind="ExternalInput").ap()
    wvT = nc.dram_tensor("wvT", [D, D], MD, kind="ExternalInput").ap()
    woTs = nc.dram_tensor("woTs", [D, D], MD, kind="ExternalInput").ap()
    m3 = nc.dram_tensor("m3", [C, C], F32, kind="ExternalInput").ap()
    qsc = nc.dram_tensor("qsc", [P, C], F32, kind="ExternalInput").ap()
    ksc = nc.dram_tensor("ksc", [P, NS], F32, kind="ExternalInput").ap()
    idn = nc.dram_tensor("idn", [P, P], MD, kind="ExternalInput").ap()
    idc = nc.dram_tensor("idc", [P, P], MD, kind="ExternalInput").ap()
    out = nc.dram_tensor("out", [NL * C, D], F32, kind="ExternalOutput").ap()

    xT_t = xT.rearrange("(eo p) t -> p eo t", p=P)          # [128, 4, NE*C]
    wq_t = wqT.rearrange("(eo p) d -> p eo d", p=P)
    wk_t = wkT.rearrange("(eo p) d -> p eo d", p=P)
    wv_t = wvT.rearrange("(eo p) d -> p eo d", p=P)
    wo_t = woTs.rearrange("(eo p) d -> p eo d", p=P)
    m3_t = m3.rearrange("(so p) t -> p so t", p=P)
    out_t = out.rearrange("(c ts p) d -> p c ts d", p=P, ts=NS)

    with tile.TileContext(nc) as tc:
        with (
            tc.tile_pool(name="wpool", bufs=1) as wpool,
            tc.tile_pool(name="cpool", bufs=1) as cpool,
            tc.tile_pool(name="state", bufs=2) as state,
            tc.tile_pool(name="proj", bufs=2) as proj,
            tc.tile_pool(name="work", bufs=2) as work,
            tc.tile_pool(name="ppa", bufs=3, space="PSUM") as ppa,
            tc.tile_pool(name="ppr", bufs=2, space="PSUM") as ppr,
            tc.tile_pool(name="ppt", bufs=2, space="PSUM") as ppt,
        ):
            mult = mybir.AluOpType.mult

            wq_sb = wpool.tile([P, NS, D], MD)
            nc.sync.dma_start(wq_sb, wq_t)
            wk_sb = wpool.tile([P, NS, D], MD)
            nc.sync.dma_start(wk_sb, wk_t)
            wv_sb = wpool.tile([P, NS, D], MD)
            nc.sync.dma_start(wv_sb, wv_t)
            wo_sb = wpool.tile([P, NS, D], MD)
            nc.sync.dma_start(wo_sb, wo_t)
            m3_sb = cpool.tile([P, NS, C], F32)
            nc.sync.dma_start(m3_sb, m3_t)
            qsc_sb = cpool.tile([P, C], F32)
            nc.sync.dma_start(qsc_sb, qsc)
            ksc_sb = cpool.tile([P, NS], F32)
            nc.sync.dma_start(ksc_sb, ksc)
            idn_sb = cpool.tile([P, P], MD)
            nc.sync.dma_start(idn_sb, idn)
            idc_sb = cpool.tile([P, P], MD)
            nc.sync.dma_start(idc_sb, idc)

            kv_prev = None   # (kscaled, v) tiles of chunk c+1
            S_prev = None    # state tile from previous (later) chunk

            for c in range(NE - 1, -1, -1):
                local = c < NL

                # ---- state update: S_c = Z_{c+1} (+ gamma^C * S_{c+1}) ----
                S_cur = None
                if kv_prev is not None and (local or has_state):
                    ksc_p, v_p = kv_prev
                    S_cur = state.tile([P, NS, D], MD, tag="S", name=f"S_{c}")
                    for eo in range(NS):
                        ps = ppa.tile([P, D], F32, tag="pa", name=f"psS_{c}_{eo}")
                        with_id = has_state and S_prev is not None
                        for so in range(NS):
                            nc.tensor.matmul(
                                ps,
                                ksc_p[:, so, eo * P:(eo + 1) * P],
                                v_p[:, so, :],
                                start=(so == 0),
                                stop=(so == NS - 1 and not with_id),
                            )
                        if with_id:
                            nc.tensor.matmul(
                                ps, idc_sb, S_prev[:, eo, :],
                                start=False, stop=True,
                            )
                        nc.any.tensor_copy(out=S_cur[:, eo, :], in_=ps)
                    S_prev = S_cur

                # ---- projections of chunk c ----
                need_kv = c > 0 or local
                xt = work.tile([P, NS, C], MD, tag="xt", name=f"xt_{c}")
                nc.sync.dma_start(xt, xT_t[:, :, c * C:(c + 1) * C])

                if need_kv:
                    ksc_c = proj.tile([P, NS, D], MD, tag="ksc", name=f"ksc_{c}")
                    v_c = proj.tile([P, NS, D], MD, tag="v", name=f"v_{c}")
                    for so in range(NS):
                        pk = ppa.tile([P, D], F32, tag="pa", name=f"psk_{c}_{so}")
                        for eo in range(NS):
                            nc.tensor.matmul(
                                pk,
                                xt[:, eo, so * P:(so + 1) * P],
                                wk_sb[:, eo, :],
                                start=(eo == 0), stop=(eo == NS - 1),
                            )
                        nc.any.tensor_tensor(
                            out=ksc_c[:, so, :], in0=pk,
                            in1=ksc_sb[:, so:so + 1].to_broadcast((P, D)),
                            op=mult,
                        )
                        pv = ppa.tile([P, D], F32, tag="pa", name=f"psv_{c}_{so}")
                        for eo in range(NS):
                            nc.tensor.matmul(
                                pv,
                                xt[:, eo, so * P:(so + 1) * P],
                                wv_sb[:, eo, :],
                                start=(eo == 0), stop=(eo == NS - 1),
                            )
                        nc.any.tensor_copy(out=v_c[:, so, :], in_=pv)

                if local:
                    # scaled q^T directly: qt[e, t] with gamma^(C-1-i) folded in
                    qt_c = work.tile([P, NS, C], MD, tag="qt", name=f"qt_{c}")
                    for eo in range(NS):
                        pq = ppa.tile([P, C], F32, tag="pa", name=f"psq_{c}_{eo}")
                        for ei in range(NS):
                            nc.tensor.matmul(
                                pq,
                                wq_sb[:, ei, eo * P:(eo + 1) * P],
                                xt[:, ei, :],
                                start=(ei == 0), stop=(ei == NS - 1),
                            )
                        nc.any.tensor_tensor(
                            out=qt_c[:, eo, :], in0=pq, in1=qsc_sb, op=mult,
                        )

                    # scaled k^T via PE transposes of ksc_c
                    kt_c = work.tile([P, NS, C], MD, tag="kt", name=f"kt_{c}")
                    for so in range(NS):
                        for do in range(NS):
                            pt = ppt.tile([P, P], MD, tag="pt",
                                          name=f"pst_{c}_{so}_{do}")
                            nc.tensor.transpose(
                                pt, ksc_c[:, so, do * P:(do + 1) * P], idn_sb)
                            nc.any.tensor_copy(
                                out=kt_c[:, do, so * P:(so + 1) * P], in_=pt)

                    # scores^T (both-scaled) then constant decay mask
                    at_c = work.tile([P, NS, C], MD, tag="at", name=f"at_{c}")
                    for so in range(NS):
                        psc = ppa.tile([P, C], F32, tag="pa", name=f"pssc_{c}_{so}")
                        for do in range(NS):
                            nc.tensor.matmul(
                                psc,
                                kt_c[:, do, so * P:(so + 1) * P],
                                qt_c[:, do, :],
                                start=(do == 0), stop=(do == NS - 1),
                            )
                        nc.any.tensor_tensor(
                            out=at_c[:, so, :], in0=psc, in1=m3_sb[:, so, :],
                            op=mult,
                        )

                    # retrieved^T = S^T qtil (cross) + v^T A^T (intra)
                    rt_c = work.tile([P, NS, C], MD, tag="rt", name=f"rt_{c}")
                    for do in range(NS):
                        pr = ppr.tile([P, C], F32, tag="pr", name=f"psr_{c}_{do}")
                        for eo in range(NS):
                            nc.tensor.matmul(
                                pr,
                                S_cur[:, eo, do * P:(do + 1) * P],
                                qt_c[:, eo, :],
                                start=(eo == 0), stop=False,
                            )
                        for so in range(NS):
                            nc.tensor.matmul(
                                pr,
                                v_c[:, so, do * P:(do + 1) * P],
                                at_c[:, so, :],
                                start=False, stop=(so == NS - 1),
                            )
                        nc.any.tensor_copy(out=rt_c[:, do, :], in_=pr)

                    # output projection
                    o_sb = work.tile([P, NS, D], F32, tag="o", bufs=1,
                                     name=f"o_{c}")
                    for ts in range(NS):
                        po = ppa.tile([P, D], F32, tag="pa", name=f"pso_{c}_{ts}")
                        for do in range(NS):
                            nc.tensor.matmul(
                                po,
                                rt_c[:, do, ts * P:(ts + 1) * P],
                                wo_sb[:, do, :],
                                start=(do == 0), stop=(do == NS - 1),
                            )
                        nc.any.tensor_copy(out=o_sb[:, ts, :], in_=po)
                    nc.sync.dma_start(out_t[:, c, :, :], o_sb)

                if need_kv:
                    kv_prev = (ksc_c, v_c)

    nc.compile()
    _BUILD_CACHE[key] = nc
    return nc


def _host_prep(x, Wq, Wk, Wv, Wo, decay_logit, out_scale, NE):
    """Shared weights/constants + per-core xT slices."""
    x = np.ascontiguousarray(np.asarray(x, dtype=np.float32))
    gamma = float(1.0 / (1.0 + np.exp(-np.float64(np.asarray(decay_logit)))))
    osc = float(np.asarray(out_scale))

    shared = {
        "wqT": np.ascontiguousarray(np.asarray(Wq, np.float32).T),
        "wkT": np.ascontiguousarray(np.asarray(Wk, np.float32).T),
        "wvT": np.ascontiguousarray(np.asarray(Wv, np.float32).T),
        "woTs": np.ascontiguousarray(np.asarray(Wo, np.float32).T * osc),
    }
    j = np.arange(C, dtype=np.float64)
    # ksc[p, so] = gamma^(so*128 + p)
    shared["ksc"] = np.ascontiguousarray(
        (gamma ** j).astype(np.float32).reshape(NS, P).transpose(1, 0))
    shared["qsc"] = np.broadcast_to(
        (gamma ** (C - 1 - j)).astype(np.float32)[None, :], (P, C)).copy()
    jj, ii = np.meshgrid(j, j, indexing="ij")
    m3v = np.where(jj > ii, gamma ** (-C), 0.0).astype(np.float32)
    shared["m3"] = m3v
    shared["idn"] = np.eye(P, dtype=np.float32)
    shared["idc"] = (np.eye(P) * (gamma ** C)).astype(np.float32)

    T_ext = NE * C
    in_maps = []
    for core in range(N_CORES):
        b, h = divmod(core, 2)
        start = h * (NL * C)
        xe = np.zeros((T_ext, D), np.float32)
        avail = min(T_ext, T - start)
        xe[:avail] = x[b, start:start + avail]
        m = dict(shared)
        m["xT"] = np.ascontiguousarray(xe.T)
        in_maps.append(m)
    return gamma, in_maps


def kernel(x, Wq, Wk, Wv, Wo, decay_logit, out_scale):
    global LAST_RESULTS
    gamma = float(1.0 / (1.0 + np.exp(-np.float64(np.asarray(decay_logit)))))
    fast = gamma ** C < 1e-8
    NE, has_state = (NL + 1, False) if fast else (T // C, True)

    nc = _build(NE, has_state)
    _, in_maps = _host_prep(x, Wq, Wk, Wv, Wo, decay_logit, out_scale, NE)

    res = run_bass_kernel_spmd(
        nc, in_maps, core_ids=list(range(N_CORES)), trace=False,
    )
    LAST_RESULTS = res

    result = np.zeros((B, T, D), np.float32)
    for core in range(N_CORES):
        b, h = divmod(core, 2)
        start = h * (NL * C)
        result[b, start:start + NL * C] = res.results[core]["out"]
    return result


# ---------------------------------------------------------------------------
# Benchmarking (dev-only; not used by the grading path).
# Chains `loop` sequential NEFF executions inside one jitted program (the
# bass_exec primitive is effectful, so XLA neither CSEs nor DCEs repeats) and
# reports the per-execution slope, which cancels tunnel/dispatch overhead.
# ---------------------------------------------------------------------------

def _timed_exec(nc, in_maps, loop: int) -> float:
    """Seconds of wall time for one jitted call with `loop` chained execs."""
    import time

    import jax
    from jax.sharding import Mesh, PartitionSpec
    from jax.experimental.shard_map import shard_map
    from concourse import bass2jax, mybir as _mybir

    n_cores = len(in_maps)
    partition_name = (nc.partition_id_tensor.name
                      if nc.partition_id_tensor else None)
    in_names, out_names, out_avals, zero_outs = [], [], [], []
    for alloc in nc.m.functions[0].allocations:
        if not isinstance(alloc, _mybir.MemoryLocationSet):
            continue
        name = alloc.memorylocations[0].name
        if alloc.kind == "ExternalInput":
            if name != partition_name:
                in_names.append(name)
        elif alloc.kind == "ExternalOutput":
            out_names.append(name)
            shape = tuple(alloc.tensor_shape)
            np_dt = _mybir.dt.np(alloc.dtype)
            out_avals.append(jax.core.ShapedArray(shape, np_dt))
            zero_outs.append(np.zeros(shape, np_dt))

    n_params = len(in_names)
    all_names = in_names + out_names
    if partition_name is not None:
        all_names = all_names + [partition_name]

    def _body(*args):
        outs = None
        for _ in range(loop):
            operands = list(args)
            if partition_name is not None:
                operands.append(bass2jax.partition_id_tensor())
            outs = bass2jax._bass_exec_p.bind(
                *operands,
                out_avals=tuple(out_avals),
                in_names=tuple(all_names),
                out_names=tuple(out_names),
                lowering_input_output_aliases=(),
                sim_require_finite=True,
                sim_require_nnan=True,
                nc=nc,
            )
        return tuple(outs)

    devices = jax.devices()[:n_cores]
    mesh = Mesh(np.asarray(devices), ("core",))
    n_args = n_params + len(out_names)
    sharded = jax.jit(shard_map(
        _body, mesh=mesh,
        in_specs=(PartitionSpec("core"),) * n_args,
        out_specs=(PartitionSpec("core"),) * len(out_names),
        check_rep=False,
    ), keep_unused=True)

    concat_in = [
        np.concatenate([np.asarray(in_maps[c][name]) for c in range(n_cores)],
                       axis=0)
        for name in in_names
    ]
    concat_zero = [
        np.zeros((n_cores * z.shape[0], *z.shape[1:]), z.dtype)
        for z in zero_outs
    ]
    args = concat_in + concat_zero
    out = sharded(*args)  # warmup/compile
    jax.block_until_ready(out)
    best = float("inf")
    for _ in range(3):
        t0 = time.perf_counter()
        out = sharded(*args)
        jax.block_until_ready(out)
        best = min(best, time.perf_counter() - t0)
    return best


def bench_exec_ns(x, Wq, Wk, Wv, Wo, decay_logit, out_scale,
                  loops=(1, 9)) -> float:
    """Per-execution HW time in ns, via two-point slope."""
    gamma = float(1.0 / (1.0 + np.exp(-np.float64(np.asarray(decay_logit)))))
    fast = gamma ** C < 1e-8
    NE, has_state = (NL + 1, False) if fast else (T // C, True)
    nc = _build(NE, has_state)
    _, in_maps = _host_prep(x, Wq, Wk, Wv, Wo, decay_logit, out_scale, NE)
    bass2jax_times = {k: _timed_exec(nc, in_maps, k) for k in loops}
    k0, k1 = loops
    per = (bass2jax_times[k1] - bass2jax_times[k0]) / (k1 - k0)
    return per * 1e9, bass2jax_times
